# revision 18
# baseline (speedup 1.0000x reference)
"""Trainium2 Bass kernel for nn_DataTermLayer (data-term update of optical-flow).

Key observation: the reference's bilinear warp feeds *normalized* coords in
[-1, 1] straight into a pixel-space sampler, so after clipping the gather
only ever touches I1[b, 0:3, 0:3]. The whole layer reduces to elementwise
math plus 9 per-image scalars:

  t2x = u + 2*w ; t2y = v + 2*h          (pre-division coords, f32-exact)
  x   = t2x/511 - 1 ; y = t2y/511 - 1
  warped = [x>=0][y>=0] * bilinear3x3(P, x, y)
  dt    = 0.1*(I2 - warped)
  out_u = u + dt*(I1[h+1,w]-I1[h,w]) ; out_v = v + dt*(I1[h,w+1]-I1[h,w])

Structure (2e-2 rel tolerance; measured ~1e-4):
  * I1 is cast once to bf16; the row gradient comes from the idle PE as a
    +-1 bidiagonal bf16 shift-matmul into PSUM (kills the baseline's
    duplicate shifted-I1 HBM load and the DVE subtract), and the column
    gradient is a 2x-rate bf16 DVE subtract.
  * dt0 = 0.1*I2 (bf16, ACT engine).  warped is expanded EXACTLY in the
    basis (1,t2x,EX)x(1,t2y,EY), EX=relu(t2x-1022): on the bottom-right
    warp quadrant only the 4 EX/EY-free terms run full-size; the EX terms
    live only in the last ~3 columns and EY in the last ~3 rows, patched
    by tiny strip ops (the Y strip runs on partitions 96:128 where
    EY==0 rows self-cancel).  Masks are f32-exact compares vs 511 in
    pre-division space (warped == 0 wherever 2w+u < 511 or 2h+v < 511).
  * A 3-row "band" strip (rows hz..255 of all images in one tile) redoes
    the rows adjacent to the half boundary with the full chain, as in
    the baseline.
  * The flow updates run on the GpSimd engine, everything PSUM-touching
    on DVE, single-source ops on ACT; output DMAs trigger from the ACT
    queue so they never block the SP input-DMA stream.

Sharding: pure data-parallel, 4 images per core across 8 cores.
"""
import sys

sys.path.insert(0, "/opt/trn_rl_repo")

import numpy as np
import ml_dtypes

import concourse.bass as bass
import concourse.mybir as mybir
from concourse.bass_utils import run_bass_kernel_spmd
from concourse.tile import TileContext

F32 = mybir.dt.float32
BF16 = mybir.dt.bfloat16
ALU = mybir.AluOpType
ACTF = mybir.ActivationFunctionType

C1 = 511.0  # min f32 t with fl(t/511) >= 1  (verified exhaustively)


def build_nc(n_imgs: int = 4, n_rb: int = 4, wz: int = 253, hz: int = 253,
             wze: int = 509, hze: int = 509, legalize: bool = True):
    """One NeuronCore program: n_imgs images of [512, 512].

    wz/hz: first col/row where the warp can be nonzero (t2 >= 511
    reachable).  wze/hze: first col/row where EX/EY (t2 >= 1022) can be
    nonzero.
    """
    assert n_rb == 4 and 225 <= hz <= 256 and 0 < wz <= 256
    assert wz < wze <= 512 and max(hz, 480) < hze <= 512
    W = 512
    H = n_rb * 128
    NBC = 256 - hz  # band compute rows per image (rows hz..255)
    NBR = NBC + 1   # band rows loaded per image (+1 for the row-shift grad)
    WF = W - wz     # warp-math columns
    WE = W - wze    # EX strip columns
    XS = wze - wz   # EX strip offset inside the warp quadrant
    nc = bass.Bass()

    I1 = nc.dram_tensor("I1", [n_imgs, H, W], F32, kind="ExternalInput")
    I2 = nc.dram_tensor("I2", [n_imgs, H, W], F32, kind="ExternalInput")
    FL = nc.dram_tensor("FL", [n_imgs, H, W, 2], F32, kind="ExternalInput")
    NCC = 9 * n_imgs + n_rb + 10
    CC = nc.dram_tensor("CC", [128, NCC], F32, kind="ExternalInput")
    GX = nc.dram_tensor("GX", [128, 1024], F32, kind="ExternalInput")
    SM = nc.dram_tensor("SM", [128, 384], BF16, kind="ExternalInput")
    OUT = nc.dram_tensor("OUT", [n_imgs, H, W, 2], F32, kind="ExternalOutput")

    NBP = max(1, NBR * n_imgs)  # band partitions

    with TileContext(nc) as tc:
        with (
            tc.tile_pool(name="stat", bufs=1) as pstat,
            tc.tile_pool(name="pin", bufs=3) as pin,
            tc.tile_pool(name="ptmp", bufs=2) as ptmp,
            tc.tile_pool(name="pwarp", bufs=2) as pwarp,
            tc.tile_pool(name="pband", bufs=1) as pband,
            tc.tile_pool(name="pps", bufs=2, space="PSUM") as pps,
        ):
            gx2 = pstat.tile([128, 1024], F32)
            cc = pstat.tile([128, NCC], F32)
            sm = pstat.tile([128, 384], BF16)
            cmth = pstat.tile([128, 1], F32)
            nc.gpsimd.memset(cmth[:], -1022.0)

            def cC(j):  # [128,1] column of cc
                return cc[:, j : j + 1]

            # ------------ warp chain: 4 EX/EY-free terms (+EX if asked) -----
            def warp_chain(pool, tag, P, fdims, t2x, t2y, bimg, with_ex):
                """wm = -0.1*warped*[t2x>=C1][t2y>=C1] into a fresh tile.
                with_ex=False drops the EX and EY basis terms (caller must
                patch the strips where they are nonzero)."""
                cof = 9 * n_imgs + n_rb + 1

                def col(k):
                    c = cC(cof + k) if bimg is None else cC(9 * bimg + k)
                    return c[:P]

                shp = [P] + list(fdims)

                def T(nm, bufs=1):
                    return pool.tile(shp, F32, tag=f"{tag}{nm}",
                                     name=f"{tag}{nm}", bufs=bufs)

                if with_ex:
                    ex = T("ex")
                    nc.scalar.activation(ex[:], t2x, ACTF.Relu,
                                         bias=cmth[:P], scale=1.0)
                pt = T("pt")
                nc.scalar.activation(pt[:], t2x, ACTF.Identity,
                                     bias=col(0), scale=col(1))
                qt = T("qt")
                nc.scalar.activation(qt[:], t2x, ACTF.Identity,
                                     bias=col(3), scale=col(4))
                if with_ex:
                    eg = T("eg", bufs=2)
                    nc.scalar.activation(eg[:], ex[:], ACTF.Identity,
                                         bias=0.0, scale=col(2))
                    nc.vector.tensor_tensor(pt[:], pt[:], eg[:], ALU.add)
                    eg2 = T("eg", bufs=2)
                    nc.scalar.activation(eg2[:], ex[:], ACTF.Identity,
                                         bias=0.0, scale=col(5))
                    nc.vector.tensor_tensor(qt[:], qt[:], eg2[:], ALU.add)
                nc.vector.tensor_tensor(qt[:], t2y, qt[:], ALU.mult)
                nc.vector.tensor_tensor(pt[:], pt[:], qt[:], ALU.add)
                return pt

            def apply_masks(pt, t2x, t2y):
                nc.vector.scalar_tensor_tensor(pt, t2x, C1, pt,
                                               ALU.is_ge, ALU.mult)
                nc.vector.scalar_tensor_tensor(pt, t2y, C1, pt,
                                               ALU.is_ge, ALU.mult)

            def apply_masks_mm(pool, tag, pt, t2x, t2y, shp):
                mm = pool.tile(shp, F32, tag=f"{tag}mm", name=f"{tag}mm")
                nc.gpsimd.tensor_tensor(mm[:], t2x, t2y, ALU.min)
                nc.vector.scalar_tensor_tensor(pt, mm[:], C1, pt,
                                               ALU.is_ge, ALU.mult)

            # ---------------- per-image stages ------------------------------
            # Emission is software-pipelined (A=prep+top-half, B=warp chain,
            # C=bottom-half updates) so no engine queue holds image b's late
            # ops in front of image b+1's early ones.  All input-DMA
            # triggers issue first (SP queue); output triggers share SP.
            NW = n_rb * 512
            HWD = NW // 2
            st = [dict() for _ in range(n_imgs)]
            if NBC > 0:
                bi1 = pband.tile([NBP, 512], F32)
                bi1r = pband.tile([NBP, 512], F32)
                bi2 = pband.tile([NBP, 512], F32)
                bfl = pband.tile([NBP, 512, 2], F32)
                for b in range(n_imgs):
                    bsl = slice(NBR * b, NBR * (b + 1))
                    nc.gpsimd.dma_start(bi1[bsl, :], I1[b, hz : hz + NBR, :])
                    nc.gpsimd.dma_start(
                        bi1r[bsl, :], I1[b, hz + 1 : hz + 1 + NBR, :]
                    )
                    nc.gpsimd.dma_start(bi2[bsl, :], I2[b, hz : hz + NBR, :])
                    nc.gpsimd.dma_start(
                        bfl[bsl, :, :], FL[b, hz : hz + NBR, :, :]
                    )
            for b in range(n_imgs):
                s = st[b]
                s["i1"] = pin.tile([128, NW], F32, tag="i1", bufs=2,
                                   name=f"i1_{b}")
                nc.sync.dma_start(
                    s["i1"][:].rearrange("p (rb w) -> p rb w", rb=n_rb),
                    I1[b].rearrange("(rb p) w -> p rb w", p=128),
                )
                s["i2"] = pin.tile([128, NW], F32, tag="i2", bufs=4,
                                   name=f"i2_{b}")
                nc.sync.dma_start(
                    s["i2"][:].rearrange("p (rb w) -> p rb w", rb=n_rb),
                    I2[b].rearrange("(rb p) w -> p rb w", p=128),
                )
                s["fl"] = pin.tile([128, NW, 2], F32, tag="fl", bufs=4,
                                   name=f"fl_{b}")
                nc.sync.dma_start(
                    s["fl"][:].rearrange("p (rb w) c -> p rb w c", rb=n_rb),
                    FL[b].rearrange("(rb p) w c -> p rb w c", p=128),
                )
                if b == 0:
                    nc.sync.dma_start(sm[:], SM[:])
                    nc.sync.dma_start(gx2[:], GX[:])
                    nc.sync.dma_start(cc[:], CC[:])


            def emit_band():
                if NBC == 0:
                    return
                bu = bfl[:, :, 0]
                bv = bfl[:, :, 1]
                bt2x = pband.tile([NBP, 512], F32)
                nc.vector.tensor_tensor(bt2x[:], bu, gx2[:NBP, 0:512],
                                        ALU.add)
                bt2y = pband.tile([NBP, 512], F32)
                nc.scalar.activation(
                    bt2y[:], bv, ACTF.Identity,
                    bias=cC(9 * n_imgs + n_rb)[:NBP], scale=1.0,
                )
                wmB = warp_chain(pband, "bnd", NBP, [512], bt2x[:], bt2y[:],
                                 None, with_ex=True)
                apply_masks(wmB[:], bt2x[:], bt2y[:])
                bdt = pband.tile([NBP, 512], F32)
                nc.vector.scalar_tensor_tensor(bdt[:], bi2[:], 0.1, wmB[:],
                                               ALU.mult, ALU.add)
                bg1 = pband.tile([NBP, 512], F32)
                nc.vector.tensor_tensor(bg1[:], bi1r[:], bi1[:], ALU.subtract)
                bg2 = pband.tile([NBP, 512], F32)
                nc.vector.tensor_tensor(
                    bg2[:, 0:511], bi1[:, 1:512], bi1[:, 0:511], ALU.subtract
                )
                nc.gpsimd.memset(bg2[:, 511:512], 0.0)
                nc.gpsimd.tensor_tensor(bg1[:], bdt[:], bg1[:], ALU.mult)
                nc.vector.tensor_tensor(bu, bu, bg1[:], ALU.add)
                nc.gpsimd.tensor_tensor(bg2[:], bdt[:], bg2[:], ALU.mult)
                nc.vector.tensor_tensor(bv, bv, bg2[:], ALU.add)

            def emitA(b):
                s = st[b]
                i1, i2, fl = s["i1"], s["i2"], s["fl"]
                i1b = ptmp.tile([128, NW], BF16, tag="i1b")
                nc.scalar.activation(i1b[:], i1[:], ACTF.Identity, bias=0.0,
                                     scale=1.0)
                ps = pps.tile([128, NW], F32, tag="ps")
                for rb in range(n_rb):
                    dst = ps[:, rb * 512 : (rb + 1) * 512]
                    rhs = i1b[:, rb * 512 : (rb + 1) * 512]
                    if rb < n_rb - 1:
                        nc.tensor.matmul(dst, sm[:, 0:128], rhs,
                                         start=True, stop=False)
                        rhs2 = i1b[:, (rb + 1) * 512 : (rb + 2) * 512]
                        nc.tensor.matmul(dst, sm[:, 128:256], rhs2,
                                         start=False, stop=True)
                    else:
                        nc.tensor.matmul(dst, sm[:, 256:384], rhs,
                                         start=True, stop=True)
                dt0 = ptmp.tile([128, NW], BF16, tag="dt0")
                nc.scalar.activation(dt0[:], i2[:], ACTF.Identity, bias=0.0,
                                     scale=0.1)
                g2 = ptmp.tile([128, NW], BF16, tag="g2")
                nc.vector.tensor_tensor(g2[:, 0 : NW - 1], i1b[:, 1:NW],
                                        i1b[:, 0 : NW - 1], ALU.subtract)
                g2r = g2[:].rearrange("p (r w) -> p r w", r=n_rb)
                nc.gpsimd.memset(g2r[:, :, 511:512], 0.0)
                s["ps"], s["dt0"], s["g2"] = ps, dt0, g2
                # top half (rb 0,1): warp-free -> update + store now
                flu = fl[:, :, 0]
                flv2 = fl[:, :, 1]
                tp = slice(0, HWD)
                nc.vector.tensor_tensor(i2[:, tp], dt0[:, tp], ps[:, tp],
                                        ALU.mult)
                nc.gpsimd.tensor_tensor(flu[:, tp], flu[:, tp], i2[:, tp],
                                        ALU.add)
                nc.vector.tensor_tensor(g2[:, tp], dt0[:, tp], g2[:, tp],
                                        ALU.mult)
                nc.gpsimd.tensor_tensor(flv2[:, tp], flv2[:, tp], g2[:, tp],
                                        ALU.add)

            def emitP(b):
                fl = st[b]["fl"]
                flv = fl[:].rearrange("p (r w) c -> p r w c", r=n_rb)
                if NBC > 0:
                    nc.sync.dma_start(
                        flv[hz - 128 : hz - 128 + NBC, 1, :, :],
                        bfl[NBR * b : NBR * b + NBC, :, :],
                    )
                nc.sync.dma_start(
                    OUT[b, 0:256].rearrange("(rb p) w c -> p rb w c", p=128),
                    fl[:, 0:HWD, :].rearrange("p (rb w) c -> p rb w c",
                                              rb=2),
                )

            def emitB(b):
                s = st[b]
                fl, dt0 = s["fl"], s["dt0"]
                flv = fl[:].rearrange("p (r w) c -> p r w c", r=n_rb)
                ur = flv[:, 2:4, wz:, 0]
                vr = flv[:, 2:4, wz:, 1]
                dt0v = dt0[:].rearrange("p (r w) -> p r w", r=n_rb)[
                    :, 2:4, wz:
                ]
                gxf = gx2[:].rearrange("p (r w) -> p r w", r=2)[:, :, wz:]
                t2x = pwarp.tile([128, 2, WF], F32, tag="t2x")
                nc.vector.tensor_tensor(t2x[:, 0, :], ur[:, 0, :],
                                        gxf[:, 0, :], ALU.add)
                nc.gpsimd.tensor_tensor(t2x[:, 1, :], ur[:, 1, :],
                                        gxf[:, 1, :], ALU.add)
                t2y = pwarp.tile([128, 2, WF], F32, tag="t2y")
                for rbl in range(2):
                    nc.scalar.activation(
                        t2y[:, rbl, :], vr[:, rbl, :], ACTF.Identity,
                        bias=cC(9 * n_imgs + 2 + rbl), scale=1.0,
                    )
                wm = warp_chain(pwarp, "w", 128, [2, WF], t2x[:], t2y[:], b,
                                with_ex=False)

                def fcol(k):
                    return cC(9 * b + k)

                if WE > 0:
                    exs = pwarp.tile([128, 2, WE], F32, tag="exs")
                    nc.scalar.activation(exs[:], t2x[:, :, XS:], ACTF.Relu,
                                         bias=cmth[:], scale=1.0)
                    e1 = pwarp.tile([128, 2, WE], F32, tag="e1")
                    nc.scalar.activation(e1[:], t2y[:, :, XS:],
                                         ACTF.Identity, bias=fcol(2),
                                         scale=fcol(5))
                    nc.vector.tensor_tensor(e1[:], e1[:], exs[:], ALU.mult)
                    nc.vector.tensor_tensor(wm[:, :, XS:], wm[:, :, XS:],
                                            e1[:], ALU.add)
                if hze < 512:
                    eys = pwarp.tile([128, WF], F32, tag="eys")
                    nc.scalar.activation(eys[96:128, :], t2y[96:128, 1, :],
                                         ACTF.Relu, bias=cmth[96:128],
                                         scale=1.0)
                    e2 = pwarp.tile([128, WF], F32, tag="e2")
                    nc.scalar.activation(e2[96:128, :], t2x[96:128, 1, :],
                                         ACTF.Identity,
                                         bias=fcol(6)[96:128],
                                         scale=fcol(7)[96:128])
                    if WE > 0:
                        egc = pwarp.tile([128, WE], F32, tag="egc")
                        nc.scalar.activation(egc[96:128, :],
                                             exs[96:128, 1, :],
                                             ACTF.Identity, bias=0.0,
                                             scale=fcol(8)[96:128])
                        nc.vector.tensor_tensor(e2[96:128, XS:],
                                                e2[96:128, XS:],
                                                egc[96:128, :], ALU.add)
                    nc.vector.tensor_tensor(e2[96:128, :], e2[96:128, :],
                                            eys[96:128, :], ALU.mult)
                    nc.vector.tensor_tensor(wm[96:128, 1, :],
                                            wm[96:128, 1, :],
                                            e2[96:128, :], ALU.add)
                apply_masks(wm[:], t2x[:], t2y[:])
                nc.vector.tensor_tensor(dt0v, dt0v, wm[:], ALU.add)

            def emitC(b):
                s = st[b]
                i2, fl, ps, dt0, g2 = (s["i2"], s["fl"], s["ps"], s["dt0"],
                                       s["g2"])
                flu = fl[:, :, 0]
                flv2 = fl[:, :, 1]
                bt = slice(HWD, NW)
                nc.vector.tensor_tensor(i2[:, bt], dt0[:, bt], ps[:, bt],
                                        ALU.mult)
                nc.vector.tensor_tensor(g2[:, bt], dt0[:, bt], g2[:, bt],
                                        ALU.mult)
                ue = nc.vector if b == n_imgs - 1 else nc.gpsimd
                ue.tensor_tensor(flu[:, bt], flu[:, bt], i2[:, bt],
                                 ALU.add)
                nc.gpsimd.tensor_tensor(flv2[:, bt], flv2[:, bt], g2[:, bt],
                                        ALU.add)
                nc.sync.dma_start(
                    OUT[b, 256:512].rearrange("(rb p) w c -> p rb w c",
                                              p=128),
                    fl[:, bt, :].rearrange("p (rb w) c -> p rb w c", rb=2),
                )

            emitA(0)
            emit_band()
            emitA(1)
            emitP(0)
            emitB(0)
            emitC(0)
            emitA(2)
            emitP(1)
            emitB(1)
            emitC(1)
            emitA(3)
            emitP(2)
            emitB(2)
            emitC(2)
            emitP(3)
            emitB(3)
            emitC(3)
    if legalize:
        legalize_single_wait(nc)
    return nc


# ---------------------------------------------------------------------------
# Post-pass: this walrus build encodes a single sync-wait slot per TPB
# instruction. Tile's sem assignment can emit 2+ waits on one instruction;
# hoist all but the last wait onto same-engine EventSemaphore carriers placed
# immediately before it (the sequencer then waits sequentially, which is
# semantically identical).
def legalize_single_wait(nc):
    import bass_rust

    capped = {
        mybir.EngineType.Activation,
        mybir.EngineType.DVE,
        mybir.EngineType.Pool,
        mybir.EngineType.PE,
        mybir.EngineType.SP,
    }
    exempt = {"EventSemaphore", "NoOp", "TriggerDma"}
    n = 0
    for fn in nc.m.functions:
        for blk in fn.blocks:
            insts = blk.instructions  # live list
            rebuilt = []
            changed = False
            for inst in list(insts):
                si = inst.sync_info
                waits = list(si.on_wait) if si is not None else []
                if (
                    len(waits) > 1
                    and inst.engine in capped
                    and str(inst.opcode) not in exempt
                ):
                    for w in waits[:-1]:
                        ev = mybir.InstEventSemaphore(
                            name=f"waitcarrier_{inst.name}_{n}", ins=[], outs=[]
                        )
                        ev.engine = inst.engine
                        ev.sync_info = bass_rust.SyncInfo(
                            on_wait=[w], on_update=[]
                        )
                        rebuilt.append(ev)
                        n += 1
                    inst.sync_info = bass_rust.SyncInfo(
                        on_wait=[waits[-1]], on_update=list(si.on_update)
                    )
                    changed = True
                rebuilt.append(inst)
            if changed:
                insts[:] = rebuilt
    return n


def _img_consts(P3: np.ndarray) -> np.ndarray:
    """9 warp consts F[i,j] (row-major) for one image's 3x3 corner P3[y,x].

    warped = sum_ij F'[i,j]*ay_i*ax_j, ax=(1,t2x,relu(t2x-1022)),
    ay=(1,t2y,relu(t2y-1022));  F = -0.1*F'.
    """
    P = P3.astype(np.float64)
    E = np.stack([P[:, 0], P[:, 1] - P[:, 0], P[:, 2] - P[:, 1]], axis=1)
    D = np.stack([E[0], E[1] - E[0], E[2] - E[1]], axis=0)
    r = 1.0 / 511.0
    Mx = np.array([[1.0, 0.0, 0.0], [-1.0, r, -r], [0.0, 0.0, r]])
    F = -0.1 * (Mx.T @ D @ Mx)
    return F.reshape(-1).astype(np.float32)


def host_consts(I1c: np.ndarray, n_rb: int = 4, hz: int = 253) -> np.ndarray:
    """Per-image folded warp coefficients + per-partition 2*h columns.

    I1c: [n_imgs, H, W] float32.  Returns [128, 9*n_imgs + n_rb + 10] f32.
    Per image b, cols 9*b+3*i+j hold F[i,j].  Col 9n+rb: 2*(128*rb+p).
    Col 9n+n_rb: band 2*h.  Cols 9n+n_rb+1..+9: band-partition-layout
    consts (partition NBR*b+r holds image b's values).
    """
    f = np.float32
    n_imgs = I1c.shape[0]
    cc = np.zeros((128, 9 * n_imgs + n_rb + 10), dtype=np.float32)
    allc = np.zeros((n_imgs, 9), dtype=np.float32)
    for b in range(n_imgs):
        allc[b] = _img_consts(I1c[b, 0:3, 0:3])
        cc[:, 9 * b : 9 * b + 9] = allc[b][None, :]
    p = np.arange(128, dtype=np.float32)
    for rb in range(n_rb):
        cc[:, 9 * n_imgs + rb] = f(2.0) * (f(128.0 * rb) + p)
    # band columns (NBR = 257-hz rows per image)
    base = 9 * n_imgs + n_rb
    nbr = 257 - hz
    for b in range(n_imgs):
        for r in range(nbr):
            pp = nbr * b + r
            if pp < 128:
                cc[pp, base] = f(2.0) * f(hz + r)
                cc[pp, base + 1 : base + 10] = allc[b]
    return cc


def host_gx() -> np.ndarray:
    w2 = (np.float32(2.0) * np.arange(512, dtype=np.float32)).astype(np.float32)
    return np.tile(w2, (128, 2)).astype(np.float32)


def host_sm() -> np.ndarray:
    """[128, 384] bf16: cols 0:128 = shift lhsT S (S[k,m]: +1 at k=m+1,
    -1 at k=m), cols 128:256 = patch lhsT (+1 at k=0, m=127), cols
    256:384 = S with column 127 zeroed (dy row 511 must be exactly 0)."""
    sm = np.zeros((128, 384), dtype=np.float32)
    for m in range(128):
        sm[m, m] = -1.0
        if m + 1 < 128:
            sm[m + 1, m] = 1.0
    sm[0, 128 + 127] = 1.0
    sm[:, 256:384] = sm[:, 0:128]
    sm[127, 256 + 127] = 0.0
    return sm.astype(ml_dtypes.bfloat16)


_NC = None
_NC_KEY = None


def _get_nc(wz, hz, wze, hze):
    global _NC, _NC_KEY
    if _NC is None or _NC_KEY != (wz, hz, wze, hze):
        _NC = build_nc(4, 4, wz=wz, hz=hz, wze=wze, hze=hze)
        _NC_KEY = (wz, hz, wze, hze)
    return _NC


def _splits(flow):
    umax = float(max(flow[..., 0].max(), 0.0))
    vmax = float(max(flow[..., 1].max(), 0.0))
    # first col/row where 2*x + d can reach 511.0 (f32-exact threshold)
    wz = int(min(256, max(1, (511.0 - umax) // 2 + 1)))
    hz = int(min(256, max(225, (511.0 - vmax) // 2 + 1)))
    assert np.float32(2.0 * (wz - 1)) + np.float32(umax) < np.float32(511.0)
    assert np.float32(2.0 * (hz - 1)) + np.float32(vmax) < np.float32(511.0)
    # first col/row where 2*x + d can reach 1022.0 (EX/EY strips)
    wze = int(min(512, max(wz + 1, (1022.0 - umax) // 2 + 1)))
    hze = int(min(512, max(481, (1022.0 - vmax) // 2 + 1)))
    assert wze == 512 or (
        np.float32(2.0 * (wze - 1)) + np.float32(umax) < np.float32(1022.0)
    )
    assert hze == 512 or (
        np.float32(2.0 * (hze - 1)) + np.float32(vmax) < np.float32(1022.0)
    )
    return wz, hz, wze, hze


def _make_in_maps(I1, I2, flow, wz, hz, n_cores=8):
    per = I1.shape[0] // n_cores
    gx = host_gx()
    sm = host_sm()
    in_maps = []
    for c in range(n_cores):
        sl = slice(c * per, (c + 1) * per)
        i1c = np.ascontiguousarray(I1[sl, :, :, 0], dtype=np.float32)
        in_maps.append(
            {
                "I1": i1c,
                "I2": np.ascontiguousarray(I2[sl, :, :, 0], dtype=np.float32),
                "FL": np.ascontiguousarray(flow[sl], dtype=np.float32),
                "CC": host_consts(i1c, 4, hz),
                "GX": gx,
                "SM": sm,
            }
        )
    return in_maps


def run(I1, I2, flow, trace=False, **kw):
    wz, hz, wze, hze = _splits(np.asarray(flow))
    nc = _get_nc(wz, hz, wze, hze)
    in_maps = _make_in_maps(I1, I2, flow, wz, hz)
    res = run_bass_kernel_spmd(nc, in_maps, list(range(8)), trace=trace, **kw)
    out = np.concatenate([r["OUT"] for r in res.results], axis=0)
    return out, res


def kernel(I1, I2, flow):
    out, _ = run(I1, I2, flow)
    return out.astype(np.float32)


# revision 19
# speedup vs baseline: 1.1893x; 1.1893x over previous
"""Trainium2 Bass kernel for nn_DataTermLayer (data-term update of optical-flow).

Key observation: the reference's bilinear warp feeds *normalized* coords in
[-1, 1] straight into a pixel-space sampler, so after clipping the gather
only ever touches I1[b, 0:3, 0:3]. The whole layer reduces to elementwise
math plus 9 per-image scalars:

  t2x = u + 2*w ; t2y = v + 2*h          (pre-division coords, f32-exact)
  x   = t2x/511 - 1 ; y = t2y/511 - 1
  warped = [x>=0][y>=0] * bilinear3x3(P, x, y)
  dt    = 0.1*(I2 - warped)
  out_u = u + dt*(I1[h+1,w]-I1[h,w]) ; out_v = v + dt*(I1[h,w+1]-I1[h,w])

Structure (2e-2 rel tolerance; measured ~1e-4):
  * I1 is cast once to bf16; the row gradient comes from the idle PE as a
    +-1 bidiagonal bf16 shift-matmul into PSUM (kills the baseline's
    duplicate shifted-I1 HBM load and the DVE subtract), and the column
    gradient is a 2x-rate bf16 DVE subtract.
  * dt0 = 0.1*I2 (bf16, ACT engine).  warped is expanded EXACTLY in the
    basis (1,t2x,EX)x(1,t2y,EY), EX=relu(t2x-1022): on the bottom-right
    warp quadrant only the 4 EX/EY-free terms run full-size; the EX terms
    live only in the last ~3 columns and EY in the last ~3 rows, patched
    by tiny strip ops (the Y strip runs on partitions 96:128 where
    EY==0 rows self-cancel).  Masks are f32-exact compares vs 511 in
    pre-division space (warped == 0 wherever 2w+u < 511 or 2h+v < 511).
  * A 3-row "band" strip (rows hz..255 of all images in one tile) redoes
    the rows adjacent to the half boundary with the full chain, as in
    the baseline.
  * The flow updates run on the GpSimd engine, everything PSUM-touching
    on DVE, single-source ops on ACT; output DMAs trigger from the ACT
    queue so they never block the SP input-DMA stream.

Sharding: pure data-parallel, 4 images per core across 8 cores.
"""
import sys

sys.path.insert(0, "/opt/trn_rl_repo")

import numpy as np
import ml_dtypes

import concourse.bass as bass
import concourse.mybir as mybir
from concourse.bass_utils import run_bass_kernel_spmd
from concourse.tile import TileContext

F32 = mybir.dt.float32
BF16 = mybir.dt.bfloat16
ALU = mybir.AluOpType
ACTF = mybir.ActivationFunctionType

C1 = 511.0  # min f32 t with fl(t/511) >= 1  (verified exhaustively)


def build_nc(n_imgs: int = 4, n_rb: int = 4, wz: int = 253, hz: int = 253,
             wze: int = 509, hze: int = 509, legalize: bool = True):
    """One NeuronCore program: n_imgs images of [512, 512].

    wz/hz: first col/row where the warp can be nonzero (t2 >= 511
    reachable).  wze/hze: first col/row where EX/EY (t2 >= 1022) can be
    nonzero.
    """
    assert n_rb == 4 and 225 <= hz <= 256 and 0 < wz <= 256
    assert wz < wze <= 512 and max(hz, 480) < hze <= 512
    W = 512
    H = n_rb * 128
    NBC = 256 - hz  # band compute rows per image (rows hz..255)
    NBR = NBC + 1   # band rows loaded per image (+1 for the row-shift grad)
    WF = W - wz     # warp-math columns
    WE = W - wze    # EX strip columns
    XS = wze - wz   # EX strip offset inside the warp quadrant
    nc = bass.Bass()

    I1 = nc.dram_tensor("I1", [n_imgs, H, W], F32, kind="ExternalInput")
    I2 = nc.dram_tensor("I2", [n_imgs, H, W], F32, kind="ExternalInput")
    FL = nc.dram_tensor("FL", [n_imgs, H, W, 2], F32, kind="ExternalInput")
    NCC = 9 * n_imgs + n_rb + 10
    CC = nc.dram_tensor("CC", [128, NCC], F32, kind="ExternalInput")
    GX = nc.dram_tensor("GX", [128, 1024], F32, kind="ExternalInput")
    SM = nc.dram_tensor("SM", [128, 384], BF16, kind="ExternalInput")
    OUT = nc.dram_tensor("OUT", [n_imgs, H, W, 2], F32, kind="ExternalOutput")

    NBP = max(1, NBR * n_imgs)  # band partitions

    with TileContext(nc) as tc:
        with (
            tc.tile_pool(name="stat", bufs=1) as pstat,
            tc.tile_pool(name="pin", bufs=3) as pin,
            tc.tile_pool(name="ptmp", bufs=2) as ptmp,
            tc.tile_pool(name="pwarp", bufs=2) as pwarp,
            tc.tile_pool(name="pband", bufs=1) as pband,
            tc.tile_pool(name="pps", bufs=2, space="PSUM") as pps,
        ):
            gx2 = pstat.tile([128, 1024], F32)
            nc.sync.dma_start(gx2[:], GX[:])
            cc = pstat.tile([128, NCC], F32)
            nc.sync.dma_start(cc[:], CC[:])
            sm = pstat.tile([128, 384], BF16)
            nc.sync.dma_start(sm[:], SM[:])
            cmth = pstat.tile([128, 1], F32)
            nc.gpsimd.memset(cmth[:], -1022.0)

            def cC(j):  # [128,1] column of cc
                return cc[:, j : j + 1]

            # ------------ warp chain: 4 EX/EY-free terms (+EX if asked) -----
            def warp_chain(pool, tag, P, fdims, t2x, t2y, bimg, with_ex):
                """wm = -0.1*warped*[t2x>=C1][t2y>=C1] into a fresh tile.
                with_ex=False drops the EX and EY basis terms (caller must
                patch the strips where they are nonzero)."""
                cof = 9 * n_imgs + n_rb + 1

                def col(k):
                    c = cC(cof + k) if bimg is None else cC(9 * bimg + k)
                    return c[:P]

                shp = [P] + list(fdims)

                def T(nm, bufs=1):
                    return pool.tile(shp, F32, tag=f"{tag}{nm}",
                                     name=f"{tag}{nm}", bufs=bufs)

                if with_ex:
                    ex = T("ex")
                    nc.scalar.activation(ex[:], t2x, ACTF.Relu,
                                         bias=cmth[:P], scale=1.0)
                pt = T("pt")
                nc.scalar.activation(pt[:], t2x, ACTF.Identity,
                                     bias=col(0), scale=col(1))
                qt = T("qt")
                nc.scalar.activation(qt[:], t2x, ACTF.Identity,
                                     bias=col(3), scale=col(4))
                if with_ex:
                    eg = T("eg", bufs=2)
                    nc.scalar.activation(eg[:], ex[:], ACTF.Identity,
                                         bias=0.0, scale=col(2))
                    nc.vector.tensor_tensor(pt[:], pt[:], eg[:], ALU.add)
                    eg2 = T("eg", bufs=2)
                    nc.scalar.activation(eg2[:], ex[:], ACTF.Identity,
                                         bias=0.0, scale=col(5))
                    nc.vector.tensor_tensor(qt[:], qt[:], eg2[:], ALU.add)
                nc.vector.tensor_tensor(qt[:], t2y, qt[:], ALU.mult)
                nc.vector.tensor_tensor(pt[:], pt[:], qt[:], ALU.add)
                return pt

            def apply_masks(pt, t2x, t2y):
                nc.vector.scalar_tensor_tensor(pt, t2x, C1, pt,
                                               ALU.is_ge, ALU.mult)
                nc.vector.scalar_tensor_tensor(pt, t2y, C1, pt,
                                               ALU.is_ge, ALU.mult)

            def apply_masks_mm(pool, tag, pt, t2x, t2y, shp):
                mm = pool.tile(shp, F32, tag=f"{tag}mm", name=f"{tag}mm")
                nc.gpsimd.tensor_tensor(mm[:], t2x, t2y, ALU.min)
                nc.vector.scalar_tensor_tensor(pt, mm[:], C1, pt,
                                               ALU.is_ge, ALU.mult)

            # ---------------- per-image stages ------------------------------
            # Emission is software-pipelined (A=prep+top-half, B=warp chain,
            # C=bottom-half updates) so no engine queue holds image b's late
            # ops in front of image b+1's early ones.  All input-DMA
            # triggers issue first (SP queue); output triggers share SP.
            NW = n_rb * 512
            HWD = NW // 2
            st = [dict() for _ in range(n_imgs)]
            if NBC > 0:
                bi1 = pband.tile([NBP, 512], F32)
                bi1r = pband.tile([NBP, 512], F32)
                bi2 = pband.tile([NBP, 512], F32)
                bfl = pband.tile([NBP, 512, 2], F32)
                for b in range(n_imgs):
                    bsl = slice(NBR * b, NBR * (b + 1))
                    nc.gpsimd.dma_start(bi1[bsl, :], I1[b, hz : hz + NBR, :])
                    nc.gpsimd.dma_start(
                        bi1r[bsl, :], I1[b, hz + 1 : hz + 1 + NBR, :]
                    )
                    nc.gpsimd.dma_start(bi2[bsl, :], I2[b, hz : hz + NBR, :])
                    nc.gpsimd.dma_start(
                        bfl[bsl, :, :], FL[b, hz : hz + NBR, :, :]
                    )
            for b in range(n_imgs):
                s = st[b]
                s["i1"] = pin.tile([128, NW], F32, tag="i1", bufs=2,
                                   name=f"i1_{b}")
                nc.sync.dma_start(
                    s["i1"][:].rearrange("p (rb w) -> p rb w", rb=n_rb),
                    I1[b].rearrange("(rb p) w -> p rb w", p=128),
                )
                s["i2"] = pin.tile([128, NW], F32, tag="i2", bufs=4,
                                   name=f"i2_{b}")
                nc.sync.dma_start(
                    s["i2"][:].rearrange("p (rb w) -> p rb w", rb=n_rb),
                    I2[b].rearrange("(rb p) w -> p rb w", p=128),
                )
                s["fl"] = pin.tile([128, NW, 2], F32, tag="fl", bufs=4,
                                   name=f"fl_{b}")
                nc.sync.dma_start(
                    s["fl"][:].rearrange("p (rb w) c -> p rb w c", rb=n_rb),
                    FL[b].rearrange("(rb p) w c -> p rb w c", p=128),
                )


            def emit_band():
                if NBC == 0:
                    return
                bu = bfl[:, :, 0]
                bv = bfl[:, :, 1]
                bt2x = pband.tile([NBP, 512], F32)
                nc.vector.tensor_tensor(bt2x[:], bu, gx2[:NBP, 0:512],
                                        ALU.add)
                bt2y = pband.tile([NBP, 512], F32)
                nc.scalar.activation(
                    bt2y[:], bv, ACTF.Identity,
                    bias=cC(9 * n_imgs + n_rb)[:NBP], scale=1.0,
                )
                wmB = warp_chain(pband, "bnd", NBP, [512], bt2x[:], bt2y[:],
                                 None, with_ex=True)
                apply_masks(wmB[:], bt2x[:], bt2y[:])
                bdt = pband.tile([NBP, 512], F32)
                nc.vector.scalar_tensor_tensor(bdt[:], bi2[:], 0.1, wmB[:],
                                               ALU.mult, ALU.add)
                bg1 = pband.tile([NBP, 512], F32)
                nc.vector.tensor_tensor(bg1[:], bi1r[:], bi1[:], ALU.subtract)
                bg2 = pband.tile([NBP, 512], F32)
                nc.vector.tensor_tensor(
                    bg2[:, 0:511], bi1[:, 1:512], bi1[:, 0:511], ALU.subtract
                )
                nc.gpsimd.memset(bg2[:, 511:512], 0.0)
                nc.gpsimd.tensor_tensor(bg1[:], bdt[:], bg1[:], ALU.mult)
                nc.vector.tensor_tensor(bu, bu, bg1[:], ALU.add)
                nc.gpsimd.tensor_tensor(bg2[:], bdt[:], bg2[:], ALU.mult)
                nc.vector.tensor_tensor(bv, bv, bg2[:], ALU.add)

            def emitA(b):
                s = st[b]
                i1, i2, fl = s["i1"], s["i2"], s["fl"]
                i1b = ptmp.tile([128, NW], BF16, tag="i1b")
                nc.scalar.activation(i1b[:], i1[:], ACTF.Identity, bias=0.0,
                                     scale=1.0)
                ps = pps.tile([128, NW], F32, tag="ps")
                for rb in range(n_rb):
                    dst = ps[:, rb * 512 : (rb + 1) * 512]
                    rhs = i1b[:, rb * 512 : (rb + 1) * 512]
                    if rb < n_rb - 1:
                        nc.tensor.matmul(dst, sm[:, 0:128], rhs,
                                         start=True, stop=False)
                        rhs2 = i1b[:, (rb + 1) * 512 : (rb + 2) * 512]
                        nc.tensor.matmul(dst, sm[:, 128:256], rhs2,
                                         start=False, stop=True)
                    else:
                        nc.tensor.matmul(dst, sm[:, 256:384], rhs,
                                         start=True, stop=True)
                dt0 = ptmp.tile([128, NW], BF16, tag="dt0")
                nc.scalar.activation(dt0[:], i2[:], ACTF.Identity, bias=0.0,
                                     scale=0.1)
                g2 = ptmp.tile([128, NW], BF16, tag="g2")
                nc.vector.tensor_tensor(g2[:, 0 : NW - 1], i1b[:, 1:NW],
                                        i1b[:, 0 : NW - 1], ALU.subtract)
                g2r = g2[:].rearrange("p (r w) -> p r w", r=n_rb)
                nc.gpsimd.memset(g2r[:, :, 511:512], 0.0)
                s["ps"], s["dt0"], s["g2"] = ps, dt0, g2
                # top half (rb 0,1): warp-free -> update + store now
                flu = fl[:, :, 0]
                flv2 = fl[:, :, 1]
                tp = slice(0, HWD)
                nc.vector.tensor_tensor(i2[:, tp], dt0[:, tp], ps[:, tp],
                                        ALU.mult)
                nc.gpsimd.tensor_tensor(flu[:, tp], flu[:, tp], i2[:, tp],
                                        ALU.add)
                nc.vector.tensor_tensor(g2[:, tp], dt0[:, tp], g2[:, tp],
                                        ALU.mult)
                nc.gpsimd.tensor_tensor(flv2[:, tp], flv2[:, tp], g2[:, tp],
                                        ALU.add)

            def emitP(b):
                fl = st[b]["fl"]
                flv = fl[:].rearrange("p (r w) c -> p r w c", r=n_rb)
                if NBC > 0:
                    nc.sync.dma_start(
                        flv[hz - 128 : hz - 128 + NBC, 1, :, :],
                        bfl[NBR * b : NBR * b + NBC, :, :],
                    )
                nc.sync.dma_start(
                    OUT[b, 0:256].rearrange("(rb p) w c -> p rb w c", p=128),
                    fl[:, 0:HWD, :].rearrange("p (rb w) c -> p rb w c",
                                              rb=2),
                )

            def emitB(b):
                s = st[b]
                fl, dt0 = s["fl"], s["dt0"]
                flv = fl[:].rearrange("p (r w) c -> p r w c", r=n_rb)
                ur = flv[:, 2:4, wz:, 0]
                vr = flv[:, 2:4, wz:, 1]
                dt0v = dt0[:].rearrange("p (r w) -> p r w", r=n_rb)[
                    :, 2:4, wz:
                ]
                gxf = gx2[:].rearrange("p (r w) -> p r w", r=2)[:, :, wz:]
                t2x = pwarp.tile([128, 2, WF], F32, tag="t2x")
                nc.vector.tensor_tensor(t2x[:], ur, gxf, ALU.add)
                t2y = pwarp.tile([128, 2, WF], F32, tag="t2y")
                for rbl in range(2):
                    nc.scalar.activation(
                        t2y[:, rbl, :], vr[:, rbl, :], ACTF.Identity,
                        bias=cC(9 * n_imgs + 2 + rbl), scale=1.0,
                    )
                wm = warp_chain(pwarp, "w", 128, [2, WF], t2x[:], t2y[:], b,
                                with_ex=False)

                def fcol(k):
                    return cC(9 * b + k)

                if WE > 0:
                    exs = pwarp.tile([128, 2, WE], F32, tag="exs")
                    nc.scalar.activation(exs[:], t2x[:, :, XS:], ACTF.Relu,
                                         bias=cmth[:], scale=1.0)
                    e1 = pwarp.tile([128, 2, WE], F32, tag="e1")
                    nc.scalar.activation(e1[:], t2y[:, :, XS:],
                                         ACTF.Identity, bias=fcol(2),
                                         scale=fcol(5))
                    nc.vector.tensor_tensor(e1[:], e1[:], exs[:], ALU.mult)
                    nc.vector.tensor_tensor(wm[:, :, XS:], wm[:, :, XS:],
                                            e1[:], ALU.add)
                if hze < 512:
                    eys = pwarp.tile([128, WF], F32, tag="eys")
                    nc.scalar.activation(eys[96:128, :], t2y[96:128, 1, :],
                                         ACTF.Relu, bias=cmth[96:128],
                                         scale=1.0)
                    e2 = pwarp.tile([128, WF], F32, tag="e2")
                    nc.scalar.activation(e2[96:128, :], t2x[96:128, 1, :],
                                         ACTF.Identity,
                                         bias=fcol(6)[96:128],
                                         scale=fcol(7)[96:128])
                    if WE > 0:
                        egc = pwarp.tile([128, WE], F32, tag="egc")
                        nc.scalar.activation(egc[96:128, :],
                                             exs[96:128, 1, :],
                                             ACTF.Identity, bias=0.0,
                                             scale=fcol(8)[96:128])
                        nc.vector.tensor_tensor(e2[96:128, XS:],
                                                e2[96:128, XS:],
                                                egc[96:128, :], ALU.add)
                    nc.vector.tensor_tensor(e2[96:128, :], e2[96:128, :],
                                            eys[96:128, :], ALU.mult)
                    nc.vector.tensor_tensor(wm[96:128, 1, :],
                                            wm[96:128, 1, :],
                                            e2[96:128, :], ALU.add)
                apply_masks(wm[:], t2x[:], t2y[:])
                nc.vector.tensor_tensor(dt0v, dt0v, wm[:], ALU.add)

            def emitC(b):
                s = st[b]
                i2, fl, ps, dt0, g2 = (s["i2"], s["fl"], s["ps"], s["dt0"],
                                       s["g2"])
                flu = fl[:, :, 0]
                flv2 = fl[:, :, 1]
                bt = slice(HWD, NW)
                nc.vector.tensor_tensor(i2[:, bt], dt0[:, bt], ps[:, bt],
                                        ALU.mult)
                nc.vector.tensor_tensor(g2[:, bt], dt0[:, bt], g2[:, bt],
                                        ALU.mult)
                ue = nc.vector if b == n_imgs - 1 else nc.gpsimd
                ue.tensor_tensor(flu[:, bt], flu[:, bt], i2[:, bt],
                                 ALU.add)
                nc.gpsimd.tensor_tensor(flv2[:, bt], flv2[:, bt], g2[:, bt],
                                        ALU.add)
                nc.sync.dma_start(
                    OUT[b, 256:512].rearrange("(rb p) w c -> p rb w c",
                                              p=128),
                    fl[:, bt, :].rearrange("p (rb w) c -> p rb w c", rb=2),
                )

            emitA(0)
            emit_band()
            emitA(1)
            emitP(0)
            emitB(0)
            emitC(0)
            emitA(2)
            emitP(1)
            emitB(1)
            emitC(1)
            emitA(3)
            emitP(2)
            emitB(2)
            emitC(2)
            emitP(3)
            emitB(3)
            emitC(3)
    if legalize:
        legalize_single_wait(nc)
    return nc


# ---------------------------------------------------------------------------
# Post-pass: this walrus build encodes a single sync-wait slot per TPB
# instruction. Tile's sem assignment can emit 2+ waits on one instruction;
# hoist all but the last wait onto same-engine EventSemaphore carriers placed
# immediately before it (the sequencer then waits sequentially, which is
# semantically identical).
def legalize_single_wait(nc):
    import bass_rust

    capped = {
        mybir.EngineType.Activation,
        mybir.EngineType.DVE,
        mybir.EngineType.Pool,
        mybir.EngineType.PE,
        mybir.EngineType.SP,
    }
    exempt = {"EventSemaphore", "NoOp", "TriggerDma"}
    n = 0
    for fn in nc.m.functions:
        for blk in fn.blocks:
            insts = blk.instructions  # live list
            rebuilt = []
            changed = False
            for inst in list(insts):
                si = inst.sync_info
                waits = list(si.on_wait) if si is not None else []
                if (
                    len(waits) > 1
                    and inst.engine in capped
                    and str(inst.opcode) not in exempt
                ):
                    for w in waits[:-1]:
                        ev = mybir.InstEventSemaphore(
                            name=f"waitcarrier_{inst.name}_{n}", ins=[], outs=[]
                        )
                        ev.engine = inst.engine
                        ev.sync_info = bass_rust.SyncInfo(
                            on_wait=[w], on_update=[]
                        )
                        rebuilt.append(ev)
                        n += 1
                    inst.sync_info = bass_rust.SyncInfo(
                        on_wait=[waits[-1]], on_update=list(si.on_update)
                    )
                    changed = True
                rebuilt.append(inst)
            if changed:
                insts[:] = rebuilt
    return n


def _img_consts(P3: np.ndarray) -> np.ndarray:
    """9 warp consts F[i,j] (row-major) for one image's 3x3 corner P3[y,x].

    warped = sum_ij F'[i,j]*ay_i*ax_j, ax=(1,t2x,relu(t2x-1022)),
    ay=(1,t2y,relu(t2y-1022));  F = -0.1*F'.
    """
    P = P3.astype(np.float64)
    E = np.stack([P[:, 0], P[:, 1] - P[:, 0], P[:, 2] - P[:, 1]], axis=1)
    D = np.stack([E[0], E[1] - E[0], E[2] - E[1]], axis=0)
    r = 1.0 / 511.0
    Mx = np.array([[1.0, 0.0, 0.0], [-1.0, r, -r], [0.0, 0.0, r]])
    F = -0.1 * (Mx.T @ D @ Mx)
    return F.reshape(-1).astype(np.float32)


def host_consts(I1c: np.ndarray, n_rb: int = 4, hz: int = 253) -> np.ndarray:
    """Per-image folded warp coefficients + per-partition 2*h columns.

    I1c: [n_imgs, H, W] float32.  Returns [128, 9*n_imgs + n_rb + 10] f32.
    Per image b, cols 9*b+3*i+j hold F[i,j].  Col 9n+rb: 2*(128*rb+p).
    Col 9n+n_rb: band 2*h.  Cols 9n+n_rb+1..+9: band-partition-layout
    consts (partition NBR*b+r holds image b's values).
    """
    f = np.float32
    n_imgs = I1c.shape[0]
    cc = np.zeros((128, 9 * n_imgs + n_rb + 10), dtype=np.float32)
    allc = np.zeros((n_imgs, 9), dtype=np.float32)
    for b in range(n_imgs):
        allc[b] = _img_consts(I1c[b, 0:3, 0:3])
        cc[:, 9 * b : 9 * b + 9] = allc[b][None, :]
    p = np.arange(128, dtype=np.float32)
    for rb in range(n_rb):
        cc[:, 9 * n_imgs + rb] = f(2.0) * (f(128.0 * rb) + p)
    # band columns (NBR = 257-hz rows per image)
    base = 9 * n_imgs + n_rb
    nbr = 257 - hz
    for b in range(n_imgs):
        for r in range(nbr):
            pp = nbr * b + r
            if pp < 128:
                cc[pp, base] = f(2.0) * f(hz + r)
                cc[pp, base + 1 : base + 10] = allc[b]
    return cc


def host_gx() -> np.ndarray:
    w2 = (np.float32(2.0) * np.arange(512, dtype=np.float32)).astype(np.float32)
    return np.tile(w2, (128, 2)).astype(np.float32)


def host_sm() -> np.ndarray:
    """[128, 384] bf16: cols 0:128 = shift lhsT S (S[k,m]: +1 at k=m+1,
    -1 at k=m), cols 128:256 = patch lhsT (+1 at k=0, m=127), cols
    256:384 = S with column 127 zeroed (dy row 511 must be exactly 0)."""
    sm = np.zeros((128, 384), dtype=np.float32)
    for m in range(128):
        sm[m, m] = -1.0
        if m + 1 < 128:
            sm[m + 1, m] = 1.0
    sm[0, 128 + 127] = 1.0
    sm[:, 256:384] = sm[:, 0:128]
    sm[127, 256 + 127] = 0.0
    return sm.astype(ml_dtypes.bfloat16)


_NC = None
_NC_KEY = None


def _get_nc(wz, hz, wze, hze):
    global _NC, _NC_KEY
    if _NC is None or _NC_KEY != (wz, hz, wze, hze):
        _NC = build_nc(4, 4, wz=wz, hz=hz, wze=wze, hze=hze)
        _NC_KEY = (wz, hz, wze, hze)
    return _NC


def _splits(flow):
    umax = float(max(flow[..., 0].max(), 0.0))
    vmax = float(max(flow[..., 1].max(), 0.0))
    # first col/row where 2*x + d can reach 511.0 (f32-exact threshold)
    wz = int(min(256, max(1, (511.0 - umax) // 2 + 1)))
    hz = int(min(256, max(225, (511.0 - vmax) // 2 + 1)))
    assert np.float32(2.0 * (wz - 1)) + np.float32(umax) < np.float32(511.0)
    assert np.float32(2.0 * (hz - 1)) + np.float32(vmax) < np.float32(511.0)
    # first col/row where 2*x + d can reach 1022.0 (EX/EY strips)
    wze = int(min(512, max(wz + 1, (1022.0 - umax) // 2 + 1)))
    hze = int(min(512, max(481, (1022.0 - vmax) // 2 + 1)))
    assert wze == 512 or (
        np.float32(2.0 * (wze - 1)) + np.float32(umax) < np.float32(1022.0)
    )
    assert hze == 512 or (
        np.float32(2.0 * (hze - 1)) + np.float32(vmax) < np.float32(1022.0)
    )
    return wz, hz, wze, hze


def _make_in_maps(I1, I2, flow, wz, hz, n_cores=8):
    per = I1.shape[0] // n_cores
    gx = host_gx()
    sm = host_sm()
    in_maps = []
    for c in range(n_cores):
        sl = slice(c * per, (c + 1) * per)
        i1c = np.ascontiguousarray(I1[sl, :, :, 0], dtype=np.float32)
        in_maps.append(
            {
                "I1": i1c,
                "I2": np.ascontiguousarray(I2[sl, :, :, 0], dtype=np.float32),
                "FL": np.ascontiguousarray(flow[sl], dtype=np.float32),
                "CC": host_consts(i1c, 4, hz),
                "GX": gx,
                "SM": sm,
            }
        )
    return in_maps


def run(I1, I2, flow, trace=False, **kw):
    wz, hz, wze, hze = _splits(np.asarray(flow))
    nc = _get_nc(wz, hz, wze, hze)
    in_maps = _make_in_maps(I1, I2, flow, wz, hz)
    res = run_bass_kernel_spmd(nc, in_maps, list(range(8)), trace=trace, **kw)
    out = np.concatenate([r["OUT"] for r in res.results], axis=0)
    return out, res


def kernel(I1, I2, flow):
    out, _ = run(I1, I2, flow)
    return out.astype(np.float32)


# revision 20
# speedup vs baseline: 1.2148x; 1.0215x over previous
"""Trainium2 Bass kernel for nn_DataTermLayer (data-term update of optical-flow).

Key observation: the reference's bilinear warp feeds *normalized* coords in
[-1, 1] straight into a pixel-space sampler, so after clipping the gather
only ever touches I1[b, 0:3, 0:3]. The whole layer reduces to elementwise
math plus 9 per-image scalars:

  t2x = u + 2*w ; t2y = v + 2*h          (pre-division coords, f32-exact)
  x   = t2x/511 - 1 ; y = t2y/511 - 1
  warped = [x>=0][y>=0] * bilinear3x3(P, x, y)
  dt    = 0.1*(I2 - warped)
  out_u = u + dt*(I1[h+1,w]-I1[h,w]) ; out_v = v + dt*(I1[h,w+1]-I1[h,w])

Structure (2e-2 rel tolerance; measured ~1e-4):
  * I1 is cast once to bf16; the row gradient comes from the idle PE as a
    +-1 bidiagonal bf16 shift-matmul into PSUM (kills the baseline's
    duplicate shifted-I1 HBM load and the DVE subtract), and the column
    gradient is a 2x-rate bf16 DVE subtract.
  * dt0 = 0.1*I2 (bf16, ACT engine).  warped is expanded EXACTLY in the
    basis (1,t2x,EX)x(1,t2y,EY), EX=relu(t2x-1022): on the bottom-right
    warp quadrant only the 4 EX/EY-free terms run full-size; the EX terms
    live only in the last ~3 columns and EY in the last ~3 rows, patched
    by tiny strip ops (the Y strip runs on partitions 96:128 where
    EY==0 rows self-cancel).  Masks are f32-exact compares vs 511 in
    pre-division space (warped == 0 wherever 2w+u < 511 or 2h+v < 511).
  * A 3-row "band" strip (rows hz..255 of all images in one tile) redoes
    the rows adjacent to the half boundary with the full chain, as in
    the baseline.
  * The flow updates run on the GpSimd engine, everything PSUM-touching
    on DVE, single-source ops on ACT; output DMAs trigger from the ACT
    queue so they never block the SP input-DMA stream.

Sharding: pure data-parallel, 4 images per core across 8 cores.
"""
import sys

sys.path.insert(0, "/opt/trn_rl_repo")

import numpy as np
import ml_dtypes

import concourse.bass as bass
import concourse.mybir as mybir
from concourse.bass_utils import run_bass_kernel_spmd
from concourse.tile import TileContext

F32 = mybir.dt.float32
BF16 = mybir.dt.bfloat16
ALU = mybir.AluOpType
ACTF = mybir.ActivationFunctionType

C1 = 511.0  # min f32 t with fl(t/511) >= 1  (verified exhaustively)


def build_nc(n_imgs: int = 4, n_rb: int = 4, wz: int = 253, hz: int = 253,
             wze: int = 509, hze: int = 509, legalize: bool = True):
    """One NeuronCore program: n_imgs images of [512, 512].

    wz/hz: first col/row where the warp can be nonzero (t2 >= 511
    reachable).  wze/hze: first col/row where EX/EY (t2 >= 1022) can be
    nonzero.
    """
    assert n_rb == 4 and 225 <= hz <= 256 and 0 < wz <= 256
    assert wz < wze <= 512 and max(hz, 480) < hze <= 512
    W = 512
    H = n_rb * 128
    NBC = 256 - hz  # band compute rows per image (rows hz..255)
    NBR = NBC + 1   # band rows loaded per image (+1 for the row-shift grad)
    WF = W - wz     # warp-math columns
    WE = W - wze    # EX strip columns
    XS = wze - wz   # EX strip offset inside the warp quadrant
    nc = bass.Bass()

    I1 = nc.dram_tensor("I1", [n_imgs, H, W], F32, kind="ExternalInput")
    I2 = nc.dram_tensor("I2", [n_imgs, H, W], F32, kind="ExternalInput")
    FL = nc.dram_tensor("FL", [n_imgs, H, W, 2], F32, kind="ExternalInput")
    NCC = 9 * n_imgs + n_rb + 10
    CC = nc.dram_tensor("CC", [128, NCC], F32, kind="ExternalInput")
    GX = nc.dram_tensor("GX", [128, 1024], F32, kind="ExternalInput")
    SM = nc.dram_tensor("SM", [128, 384], BF16, kind="ExternalInput")
    OUT = nc.dram_tensor("OUT", [n_imgs, H, W, 2], F32, kind="ExternalOutput")

    NBP = max(1, NBR * n_imgs)  # band partitions

    with TileContext(nc) as tc:
        with (
            tc.tile_pool(name="stat", bufs=1) as pstat,
            tc.tile_pool(name="pin", bufs=3) as pin,
            tc.tile_pool(name="ptmp", bufs=2) as ptmp,
            tc.tile_pool(name="pwarp", bufs=2) as pwarp,
            tc.tile_pool(name="pband", bufs=1) as pband,
            tc.tile_pool(name="pps", bufs=2, space="PSUM") as pps,
        ):
            gx2 = pstat.tile([128, 1024], F32)
            nc.sync.dma_start(gx2[:], GX[:])
            cc = pstat.tile([128, NCC], F32)
            nc.sync.dma_start(cc[:], CC[:])
            sm = pstat.tile([128, 384], BF16)
            nc.sync.dma_start(sm[:], SM[:])
            cmth = pstat.tile([128, 1], F32)
            nc.gpsimd.memset(cmth[:], -1022.0)

            def cC(j):  # [128,1] column of cc
                return cc[:, j : j + 1]

            # ------------ warp chain: 4 EX/EY-free terms (+EX if asked) -----
            def warp_chain(pool, tag, P, fdims, t2x, t2y, bimg, with_ex):
                """wm = -0.1*warped*[t2x>=C1][t2y>=C1] into a fresh tile.
                with_ex=False drops the EX and EY basis terms (caller must
                patch the strips where they are nonzero)."""
                cof = 9 * n_imgs + n_rb + 1

                def col(k):
                    c = cC(cof + k) if bimg is None else cC(9 * bimg + k)
                    return c[:P]

                shp = [P] + list(fdims)

                def T(nm, bufs=1):
                    return pool.tile(shp, F32, tag=f"{tag}{nm}",
                                     name=f"{tag}{nm}", bufs=bufs)

                if with_ex:
                    ex = T("ex")
                    nc.scalar.activation(ex[:], t2x, ACTF.Relu,
                                         bias=cmth[:P], scale=1.0)
                pt = T("pt")
                nc.scalar.activation(pt[:], t2x, ACTF.Identity,
                                     bias=col(0), scale=col(1))
                qt = T("qt")
                nc.scalar.activation(qt[:], t2x, ACTF.Identity,
                                     bias=col(3), scale=col(4))
                if with_ex:
                    eg = T("eg", bufs=2)
                    nc.scalar.activation(eg[:], ex[:], ACTF.Identity,
                                         bias=0.0, scale=col(2))
                    nc.vector.tensor_tensor(pt[:], pt[:], eg[:], ALU.add)
                    eg2 = T("eg", bufs=2)
                    nc.scalar.activation(eg2[:], ex[:], ACTF.Identity,
                                         bias=0.0, scale=col(5))
                    nc.vector.tensor_tensor(qt[:], qt[:], eg2[:], ALU.add)
                nc.vector.tensor_tensor(qt[:], t2y, qt[:], ALU.mult)
                nc.vector.tensor_tensor(pt[:], pt[:], qt[:], ALU.add)
                return pt

            def apply_masks(pt, t2x, t2y):
                nc.vector.scalar_tensor_tensor(pt, t2x, C1, pt,
                                               ALU.is_ge, ALU.mult)
                nc.vector.scalar_tensor_tensor(pt, t2y, C1, pt,
                                               ALU.is_ge, ALU.mult)

            def apply_masks_mm(pool, tag, pt, t2x, t2y, shp):
                mm = pool.tile(shp, F32, tag=f"{tag}mm", name=f"{tag}mm")
                nc.gpsimd.tensor_tensor(mm[:], t2x, t2y, ALU.min)
                nc.vector.scalar_tensor_tensor(pt, mm[:], C1, pt,
                                               ALU.is_ge, ALU.mult)

            # ---------------- per-image stages ------------------------------
            # Emission is software-pipelined (A=prep+top-half, B=warp chain,
            # C=bottom-half updates) so no engine queue holds image b's late
            # ops in front of image b+1's early ones.  All input-DMA
            # triggers issue first (SP queue); output triggers share SP.
            NW = n_rb * 512
            HWD = NW // 2
            st = [dict() for _ in range(n_imgs)]
            if NBC > 0:
                bi1 = pband.tile([NBP, 512], F32)
                bi1r = pband.tile([NBP, 512], F32)
                bi2 = pband.tile([NBP, 512], F32)
                bfl = pband.tile([NBP, 512, 2], F32)
                for b in range(n_imgs):
                    bsl = slice(NBR * b, NBR * (b + 1))
                    nc.gpsimd.dma_start(bi1[bsl, :], I1[b, hz : hz + NBR, :])
                    nc.gpsimd.dma_start(
                        bi1r[bsl, :], I1[b, hz + 1 : hz + 1 + NBR, :]
                    )
                    nc.gpsimd.dma_start(bi2[bsl, :], I2[b, hz : hz + NBR, :])
                    nc.gpsimd.dma_start(
                        bfl[bsl, :, :], FL[b, hz : hz + NBR, :, :]
                    )
            for b in range(n_imgs):
                s = st[b]
                s["i1"] = pin.tile([128, NW], F32, tag="i1", bufs=3,
                                   name=f"i1_{b}")
                nc.sync.dma_start(
                    s["i1"][:].rearrange("p (rb w) -> p rb w", rb=n_rb),
                    I1[b].rearrange("(rb p) w -> p rb w", p=128),
                )
                s["i2"] = pin.tile([128, NW], F32, tag="i2", bufs=4,
                                   name=f"i2_{b}")
                nc.sync.dma_start(
                    s["i2"][:].rearrange("p (rb w) -> p rb w", rb=n_rb),
                    I2[b].rearrange("(rb p) w -> p rb w", p=128),
                )
                s["fl"] = pin.tile([128, NW, 2], F32, tag="fl", bufs=4,
                                   name=f"fl_{b}")
                nc.sync.dma_start(
                    s["fl"][:].rearrange("p (rb w) c -> p rb w c", rb=n_rb),
                    FL[b].rearrange("(rb p) w c -> p rb w c", p=128),
                )


            def emit_band():
                if NBC == 0:
                    return
                bu = bfl[:, :, 0]
                bv = bfl[:, :, 1]
                bt2x = pband.tile([NBP, 512], F32)
                nc.vector.tensor_tensor(bt2x[:], bu, gx2[:NBP, 0:512],
                                        ALU.add)
                bt2y = pband.tile([NBP, 512], F32)
                nc.scalar.activation(
                    bt2y[:], bv, ACTF.Identity,
                    bias=cC(9 * n_imgs + n_rb)[:NBP], scale=1.0,
                )
                wmB = warp_chain(pband, "bnd", NBP, [512], bt2x[:], bt2y[:],
                                 None, with_ex=True)
                apply_masks(wmB[:], bt2x[:], bt2y[:])
                bdt = pband.tile([NBP, 512], F32)
                nc.vector.scalar_tensor_tensor(bdt[:], bi2[:], 0.1, wmB[:],
                                               ALU.mult, ALU.add)
                bg1 = pband.tile([NBP, 512], F32)
                nc.vector.tensor_tensor(bg1[:], bi1r[:], bi1[:], ALU.subtract)
                bg2 = pband.tile([NBP, 512], F32)
                nc.vector.tensor_tensor(
                    bg2[:, 0:511], bi1[:, 1:512], bi1[:, 0:511], ALU.subtract
                )
                nc.gpsimd.memset(bg2[:, 511:512], 0.0)
                nc.gpsimd.tensor_tensor(bg1[:], bdt[:], bg1[:], ALU.mult)
                nc.vector.tensor_tensor(bu, bu, bg1[:], ALU.add)
                nc.gpsimd.tensor_tensor(bg2[:], bdt[:], bg2[:], ALU.mult)
                nc.vector.tensor_tensor(bv, bv, bg2[:], ALU.add)

            def emitA(b):
                s = st[b]
                i1, i2, fl = s["i1"], s["i2"], s["fl"]
                i1b = ptmp.tile([128, NW], BF16, tag="i1b", bufs=3)
                nc.scalar.activation(i1b[:], i1[:], ACTF.Identity, bias=0.0,
                                     scale=1.0)
                ps = pps.tile([128, NW], F32, tag="ps")
                for rb in range(n_rb):
                    dst = ps[:, rb * 512 : (rb + 1) * 512]
                    rhs = i1b[:, rb * 512 : (rb + 1) * 512]
                    if rb < n_rb - 1:
                        nc.tensor.matmul(dst, sm[:, 0:128], rhs,
                                         start=True, stop=False)
                        rhs2 = i1b[:, (rb + 1) * 512 : (rb + 2) * 512]
                        nc.tensor.matmul(dst, sm[:, 128:256], rhs2,
                                         start=False, stop=True)
                    else:
                        nc.tensor.matmul(dst, sm[:, 256:384], rhs,
                                         start=True, stop=True)
                dt0 = ptmp.tile([128, NW], BF16, tag="dt0", bufs=3)
                nc.scalar.activation(dt0[:], i2[:], ACTF.Identity, bias=0.0,
                                     scale=0.1)
                g2 = ptmp.tile([128, NW], BF16, tag="g2", bufs=3)
                nc.vector.tensor_tensor(g2[:, 0 : NW - 1], i1b[:, 1:NW],
                                        i1b[:, 0 : NW - 1], ALU.subtract)
                g2r = g2[:].rearrange("p (r w) -> p r w", r=n_rb)
                nc.gpsimd.memset(g2r[:, :, 511:512], 0.0)
                s["ps"], s["dt0"], s["g2"] = ps, dt0, g2
                # top half (rb 0,1): warp-free -> update + store now
                flu = fl[:, :, 0]
                flv2 = fl[:, :, 1]
                tp = slice(0, HWD)
                nc.vector.tensor_tensor(i2[:, tp], dt0[:, tp], ps[:, tp],
                                        ALU.mult)
                nc.gpsimd.tensor_tensor(flu[:, tp], flu[:, tp], i2[:, tp],
                                        ALU.add)
                nc.vector.tensor_tensor(g2[:, tp], dt0[:, tp], g2[:, tp],
                                        ALU.mult)
                nc.gpsimd.tensor_tensor(flv2[:, tp], flv2[:, tp], g2[:, tp],
                                        ALU.add)

            def emitP(b):
                fl = st[b]["fl"]
                flv = fl[:].rearrange("p (r w) c -> p r w c", r=n_rb)
                if NBC > 0:
                    nc.sync.dma_start(
                        flv[hz - 128 : hz - 128 + NBC, 1, :, :],
                        bfl[NBR * b : NBR * b + NBC, :, :],
                    )
                nc.sync.dma_start(
                    OUT[b, 0:256].rearrange("(rb p) w c -> p rb w c", p=128),
                    fl[:, 0:HWD, :].rearrange("p (rb w) c -> p rb w c",
                                              rb=2),
                )

            def emitB(b):
                s = st[b]
                fl, dt0 = s["fl"], s["dt0"]
                flv = fl[:].rearrange("p (r w) c -> p r w c", r=n_rb)
                ur = flv[:, 2:4, wz:, 0]
                vr = flv[:, 2:4, wz:, 1]
                dt0v = dt0[:].rearrange("p (r w) -> p r w", r=n_rb)[
                    :, 2:4, wz:
                ]
                gxf = gx2[:].rearrange("p (r w) -> p r w", r=2)[:, :, wz:]
                t2x = pwarp.tile([128, 2, WF], F32, tag="t2x")
                nc.vector.tensor_tensor(t2x[:], ur, gxf, ALU.add)
                t2y = pwarp.tile([128, 2, WF], F32, tag="t2y")
                for rbl in range(2):
                    nc.scalar.activation(
                        t2y[:, rbl, :], vr[:, rbl, :], ACTF.Identity,
                        bias=cC(9 * n_imgs + 2 + rbl), scale=1.0,
                    )
                wm = warp_chain(pwarp, "w", 128, [2, WF], t2x[:], t2y[:], b,
                                with_ex=False)

                def fcol(k):
                    return cC(9 * b + k)

                if WE > 0:
                    exs = pwarp.tile([128, 2, WE], F32, tag="exs")
                    nc.scalar.activation(exs[:], t2x[:, :, XS:], ACTF.Relu,
                                         bias=cmth[:], scale=1.0)
                    e1 = pwarp.tile([128, 2, WE], F32, tag="e1")
                    nc.scalar.activation(e1[:], t2y[:, :, XS:],
                                         ACTF.Identity, bias=fcol(2),
                                         scale=fcol(5))
                    nc.vector.tensor_tensor(e1[:], e1[:], exs[:], ALU.mult)
                    nc.vector.tensor_tensor(wm[:, :, XS:], wm[:, :, XS:],
                                            e1[:], ALU.add)
                if hze < 512:
                    eys = pwarp.tile([128, WF], F32, tag="eys")
                    nc.scalar.activation(eys[96:128, :], t2y[96:128, 1, :],
                                         ACTF.Relu, bias=cmth[96:128],
                                         scale=1.0)
                    e2 = pwarp.tile([128, WF], F32, tag="e2")
                    nc.scalar.activation(e2[96:128, :], t2x[96:128, 1, :],
                                         ACTF.Identity,
                                         bias=fcol(6)[96:128],
                                         scale=fcol(7)[96:128])
                    if WE > 0:
                        egc = pwarp.tile([128, WE], F32, tag="egc")
                        nc.scalar.activation(egc[96:128, :],
                                             exs[96:128, 1, :],
                                             ACTF.Identity, bias=0.0,
                                             scale=fcol(8)[96:128])
                        nc.vector.tensor_tensor(e2[96:128, XS:],
                                                e2[96:128, XS:],
                                                egc[96:128, :], ALU.add)
                    nc.vector.tensor_tensor(e2[96:128, :], e2[96:128, :],
                                            eys[96:128, :], ALU.mult)
                    nc.vector.tensor_tensor(wm[96:128, 1, :],
                                            wm[96:128, 1, :],
                                            e2[96:128, :], ALU.add)
                apply_masks(wm[:], t2x[:], t2y[:])
                nc.vector.tensor_tensor(dt0v, dt0v, wm[:], ALU.add)

            def emitC(b):
                s = st[b]
                i2, fl, ps, dt0, g2 = (s["i2"], s["fl"], s["ps"], s["dt0"],
                                       s["g2"])
                flu = fl[:, :, 0]
                flv2 = fl[:, :, 1]
                bt = slice(HWD, NW)
                nc.vector.tensor_tensor(i2[:, bt], dt0[:, bt], ps[:, bt],
                                        ALU.mult)
                nc.vector.tensor_tensor(g2[:, bt], dt0[:, bt], g2[:, bt],
                                        ALU.mult)
                ue = nc.vector if b == n_imgs - 1 else nc.gpsimd
                ue.tensor_tensor(flu[:, bt], flu[:, bt], i2[:, bt],
                                 ALU.add)
                nc.gpsimd.tensor_tensor(flv2[:, bt], flv2[:, bt], g2[:, bt],
                                        ALU.add)
                nc.sync.dma_start(
                    OUT[b, 256:512].rearrange("(rb p) w c -> p rb w c",
                                              p=128),
                    fl[:, bt, :].rearrange("p (rb w) c -> p rb w c", rb=2),
                )

            emitA(0)
            emit_band()
            emitA(1)
            emitP(0)
            emitB(0)
            emitC(0)
            emitA(2)
            emitP(1)
            emitB(1)
            emitC(1)
            emitA(3)
            emitP(2)
            emitB(2)
            emitC(2)
            emitP(3)
            emitB(3)
            emitC(3)
    if legalize:
        legalize_single_wait(nc)
    return nc


# ---------------------------------------------------------------------------
# Post-pass: this walrus build encodes a single sync-wait slot per TPB
# instruction. Tile's sem assignment can emit 2+ waits on one instruction;
# hoist all but the last wait onto same-engine EventSemaphore carriers placed
# immediately before it (the sequencer then waits sequentially, which is
# semantically identical).
def legalize_single_wait(nc):
    import bass_rust

    capped = {
        mybir.EngineType.Activation,
        mybir.EngineType.DVE,
        mybir.EngineType.Pool,
        mybir.EngineType.PE,
        mybir.EngineType.SP,
    }
    exempt = {"EventSemaphore", "NoOp", "TriggerDma"}
    n = 0
    for fn in nc.m.functions:
        for blk in fn.blocks:
            insts = blk.instructions  # live list
            rebuilt = []
            changed = False
            for inst in list(insts):
                si = inst.sync_info
                waits = list(si.on_wait) if si is not None else []
                if (
                    len(waits) > 1
                    and inst.engine in capped
                    and str(inst.opcode) not in exempt
                ):
                    for w in waits[:-1]:
                        ev = mybir.InstEventSemaphore(
                            name=f"waitcarrier_{inst.name}_{n}", ins=[], outs=[]
                        )
                        ev.engine = inst.engine
                        ev.sync_info = bass_rust.SyncInfo(
                            on_wait=[w], on_update=[]
                        )
                        rebuilt.append(ev)
                        n += 1
                    inst.sync_info = bass_rust.SyncInfo(
                        on_wait=[waits[-1]], on_update=list(si.on_update)
                    )
                    changed = True
                rebuilt.append(inst)
            if changed:
                insts[:] = rebuilt
    return n


def _img_consts(P3: np.ndarray) -> np.ndarray:
    """9 warp consts F[i,j] (row-major) for one image's 3x3 corner P3[y,x].

    warped = sum_ij F'[i,j]*ay_i*ax_j, ax=(1,t2x,relu(t2x-1022)),
    ay=(1,t2y,relu(t2y-1022));  F = -0.1*F'.
    """
    P = P3.astype(np.float64)
    E = np.stack([P[:, 0], P[:, 1] - P[:, 0], P[:, 2] - P[:, 1]], axis=1)
    D = np.stack([E[0], E[1] - E[0], E[2] - E[1]], axis=0)
    r = 1.0 / 511.0
    Mx = np.array([[1.0, 0.0, 0.0], [-1.0, r, -r], [0.0, 0.0, r]])
    F = -0.1 * (Mx.T @ D @ Mx)
    return F.reshape(-1).astype(np.float32)


def host_consts(I1c: np.ndarray, n_rb: int = 4, hz: int = 253) -> np.ndarray:
    """Per-image folded warp coefficients + per-partition 2*h columns.

    I1c: [n_imgs, H, W] float32.  Returns [128, 9*n_imgs + n_rb + 10] f32.
    Per image b, cols 9*b+3*i+j hold F[i,j].  Col 9n+rb: 2*(128*rb+p).
    Col 9n+n_rb: band 2*h.  Cols 9n+n_rb+1..+9: band-partition-layout
    consts (partition NBR*b+r holds image b's values).
    """
    f = np.float32
    n_imgs = I1c.shape[0]
    cc = np.zeros((128, 9 * n_imgs + n_rb + 10), dtype=np.float32)
    allc = np.zeros((n_imgs, 9), dtype=np.float32)
    for b in range(n_imgs):
        allc[b] = _img_consts(I1c[b, 0:3, 0:3])
        cc[:, 9 * b : 9 * b + 9] = allc[b][None, :]
    p = np.arange(128, dtype=np.float32)
    for rb in range(n_rb):
        cc[:, 9 * n_imgs + rb] = f(2.0) * (f(128.0 * rb) + p)
    # band columns (NBR = 257-hz rows per image)
    base = 9 * n_imgs + n_rb
    nbr = 257 - hz
    for b in range(n_imgs):
        for r in range(nbr):
            pp = nbr * b + r
            if pp < 128:
                cc[pp, base] = f(2.0) * f(hz + r)
                cc[pp, base + 1 : base + 10] = allc[b]
    return cc


def host_gx() -> np.ndarray:
    w2 = (np.float32(2.0) * np.arange(512, dtype=np.float32)).astype(np.float32)
    return np.tile(w2, (128, 2)).astype(np.float32)


def host_sm() -> np.ndarray:
    """[128, 384] bf16: cols 0:128 = shift lhsT S (S[k,m]: +1 at k=m+1,
    -1 at k=m), cols 128:256 = patch lhsT (+1 at k=0, m=127), cols
    256:384 = S with column 127 zeroed (dy row 511 must be exactly 0)."""
    sm = np.zeros((128, 384), dtype=np.float32)
    for m in range(128):
        sm[m, m] = -1.0
        if m + 1 < 128:
            sm[m + 1, m] = 1.0
    sm[0, 128 + 127] = 1.0
    sm[:, 256:384] = sm[:, 0:128]
    sm[127, 256 + 127] = 0.0
    return sm.astype(ml_dtypes.bfloat16)


_NC = None
_NC_KEY = None


def _get_nc(wz, hz, wze, hze):
    global _NC, _NC_KEY
    if _NC is None or _NC_KEY != (wz, hz, wze, hze):
        _NC = build_nc(4, 4, wz=wz, hz=hz, wze=wze, hze=hze)
        _NC_KEY = (wz, hz, wze, hze)
    return _NC


def _splits(flow):
    umax = float(max(flow[..., 0].max(), 0.0))
    vmax = float(max(flow[..., 1].max(), 0.0))
    # first col/row where 2*x + d can reach 511.0 (f32-exact threshold)
    wz = int(min(256, max(1, (511.0 - umax) // 2 + 1)))
    hz = int(min(256, max(225, (511.0 - vmax) // 2 + 1)))
    assert np.float32(2.0 * (wz - 1)) + np.float32(umax) < np.float32(511.0)
    assert np.float32(2.0 * (hz - 1)) + np.float32(vmax) < np.float32(511.0)
    # first col/row where 2*x + d can reach 1022.0 (EX/EY strips)
    wze = int(min(512, max(wz + 1, (1022.0 - umax) // 2 + 1)))
    hze = int(min(512, max(481, (1022.0 - vmax) // 2 + 1)))
    assert wze == 512 or (
        np.float32(2.0 * (wze - 1)) + np.float32(umax) < np.float32(1022.0)
    )
    assert hze == 512 or (
        np.float32(2.0 * (hze - 1)) + np.float32(vmax) < np.float32(1022.0)
    )
    return wz, hz, wze, hze


def _make_in_maps(I1, I2, flow, wz, hz, n_cores=8):
    per = I1.shape[0] // n_cores
    gx = host_gx()
    sm = host_sm()
    in_maps = []
    for c in range(n_cores):
        sl = slice(c * per, (c + 1) * per)
        i1c = np.ascontiguousarray(I1[sl, :, :, 0], dtype=np.float32)
        in_maps.append(
            {
                "I1": i1c,
                "I2": np.ascontiguousarray(I2[sl, :, :, 0], dtype=np.float32),
                "FL": np.ascontiguousarray(flow[sl], dtype=np.float32),
                "CC": host_consts(i1c, 4, hz),
                "GX": gx,
                "SM": sm,
            }
        )
    return in_maps


def run(I1, I2, flow, trace=False, **kw):
    wz, hz, wze, hze = _splits(np.asarray(flow))
    nc = _get_nc(wz, hz, wze, hze)
    in_maps = _make_in_maps(I1, I2, flow, wz, hz)
    res = run_bass_kernel_spmd(nc, in_maps, list(range(8)), trace=trace, **kw)
    out = np.concatenate([r["OUT"] for r in res.results], axis=0)
    return out, res


def kernel(I1, I2, flow):
    out, _ = run(I1, I2, flow)
    return out.astype(np.float32)
